# revision 21
# baseline (speedup 1.0000x reference)
"""Trainium2 Bass kernel for hetero GNN (2x SAGEConv layers + in/out proj).

Full inputs in, full output out. Design (v2):
- dst-node sharding across 8 cores (shard=12544 per core).
- Article input projection REPLICATED on every core (per-core column-rotated
  xaT input so each core's own shard lands at columns [0, shard)) -> local
  full article table `atab`, no input AllGathers.
- User projection sharded; u goes into cols 0:64 of a combined per-AG-chunk
  shard buffer; conv c1b writes u1 into cols 64:128. Seven chunked
  AllGathers (issued inside the c1b block loop) build the combined
  [u | u1] table `comb_rm`, hidden under c1b compute.
- conv c1p + c2p FUSED: one gather pass reads 256B rows carrying both u and
  u1, S (one-hot scatter) matrices built once and used by both PSUM chains.
- Gathers chunked: CH blocks x 4 src-quadrants per dma_gather call.
"""

import math

import numpy as np

import concourse.bacc as bacc
import concourse.bass as bass
import concourse.mybir as mybir
from concourse import tile
from concourse.bass_utils import run_bass_kernel_spmd

FP32 = mybir.dt.float32
BF16 = mybir.dt.bfloat16
I16 = mybir.dt.int16
AF = mybir.ActivationFunctionType
ALU = mybir.AluOpType

BF16_NP = mybir.dt.np(BF16)


def full_cfg():
    return dict(
        N=100000,
        E=1600000,
        DA=300,
        DU=64,
        H=64,
        OUT=2,
        n_cores=8,
        shard=12544,  # 98 * 128 per-core dst shard
        cq_min=5,
        CH=7,   # dst blocks per gather chunk (must divide NBLK)
        AGC=2,  # gather chunks per AllGather chunk
    )


# ----------------------------------------------------------------------------
# Host-side edge preprocessing
# ----------------------------------------------------------------------------


def user_trow(u, cfg):
    """Table row of user u in the chunk-AllGathered combined table."""
    shard, n_cores = cfg["shard"], cfg["n_cores"]
    RPA = cfg["CH"] * cfg["AGC"] * 128  # rows per AG chunk per core
    cu = u // shard
    r = u % shard
    return (r // RPA) * (n_cores * RPA) + cu * RPA + (r % RPA)


def prep_edges(src_t_per_core, dst, cfg):
    """Bucket edges by (dst gather-chunk, src quadrant, block-in-chunk).

    src_t_per_core: [n_cores, E] table row of the source node as seen by
    each core (articles: per-core rotated; users: same for all cores).
    dst: [E] global dst id (dst-sharded by range).

    Returns (CQ, per_core list of dicts idx_w/slot_w/rval_w).
    """
    N, shard, n_cores = cfg["N"], cfg["shard"], cfg["n_cores"]
    CH = cfg["CH"]
    NPAD = n_cores * shard
    QN = NPAD // 4
    assert QN < 32768, QN
    NBLK = shard // 128
    NCHUNK = NBLK // CH
    assert NCHUNK * CH == NBLK

    dst = np.asarray(dst, dtype=np.int64)
    E = len(dst)
    deg = np.bincount(dst, minlength=N).astype(np.float64)
    recip = (1.0 / np.maximum(deg, 1.0)).astype(np.float32)

    core = dst // shard
    bl = (dst % shard) >> 7
    slot_val = (dst & 127).astype(np.float32)
    rval_val = recip[dst]
    ch = bl // CH
    bb = bl % CH

    # quadrant depends on per-core src table row; compute per core
    per_core = []
    CQ = cfg["cq_min"]
    # first pass: find global CQ
    cell_list = []
    for c in range(n_cores):
        m = core == c
        st = src_t_per_core[c][m]
        q = st // QN
        cell = (ch[m] * 4 + q) * CH + bb[m]
        cell_list.append((m, st, q, cell))
        cnts = np.bincount(cell, minlength=NCHUNK * 4 * CH)
        if len(cnts):
            CQ = max(CQ, int(math.ceil(cnts.max() / 128)))
    CB = 4 * CQ
    G = CH * CQ  # 128-edge groups per (chunk, quadrant) gather call

    for c in range(n_cores):
        m, st, q, cell = cell_list[c]
        order = np.argsort(cell, kind="stable")
        cell_s = cell[order]
        n_cells = NCHUNK * 4 * CH
        starts = np.searchsorted(cell_s, np.arange(n_cells))
        j = np.arange(len(cell_s)) - starts[cell_s]  # position within cell
        st_s = st[order]
        q_s = q[order]
        ch_s = ch[m][order]
        bb_s = bb[m][order]
        bl_s = bl[m][order]
        loc_idx = (st_s % QN).astype(np.int16)
        sv = slot_val[m][order]
        rv = rval_val[m][order]

        # gather idx array: per call (ch, q), position J = bb*CQ*128 + j,
        # wrapped into 16 partitions, replicated 8x down the partition dim.
        idx_w = np.zeros((128, NCHUNK * 4 * G * 8), dtype=np.int16)
        J = bb_s * (CQ * 128) + j
        col = (ch_s * 4 + q_s) * (G * 8) + J // 16
        row = (J % 16).astype(np.int64)
        for g in range(8):
            idx_w[row + 16 * g, col] = loc_idx
        # slot / recip one-hot source arrays: [128, NBLK*CB]
        slot_w = np.full((128, NBLK * CB), 999.0, dtype=np.float32)
        rval_w = np.zeros((128, NBLK * CB), dtype=np.float32)
        colS = bl_s * CB + q_s * CQ + j // 128
        rowS = j % 128
        slot_w[rowS, colS] = sv
        rval_w[rowS, colS] = rv
        per_core.append(dict(idx_w=idx_w, slot_w=slot_w, rval_w=rval_w))
    return CQ, per_core


def _lin_bf16(w):
    """[out,in] fp32 -> lhsT layout [in,out] bf16."""
    return np.ascontiguousarray(np.asarray(w).T).astype(BF16_NP)


def _bias_col(b):
    return np.asarray(b, np.float32).reshape(-1, 1)


# ----------------------------------------------------------------------------
# Device program
# ----------------------------------------------------------------------------


def build_program(cfg, CQp, CQb, reps=1, skip=()):
    N, DA, DU, H, OUT = cfg["N"], cfg["DA"], cfg["DU"], cfg["H"], cfg["OUT"]
    n_cores, shard = cfg["n_cores"], cfg["shard"]
    CH, AGC = cfg["CH"], cfg["AGC"]
    NPAD = n_cores * shard
    QN = NPAD // 4
    NBLK = shard // 128
    NCHUNK = NBLK // CH
    NAG = NCHUNK // AGC
    assert NAG * AGC == NCHUNK
    RPA = CH * AGC * 128  # rows per AG chunk (per core)
    BPA = CH * AGC        # blocks per AG chunk
    DA_PAD = ((DA + 15) // 16) * 16  # 304
    KA = [(k, min(128, DA_PAD - k)) for k in range(0, DA_PAD, 128)]
    TW = 512
    assert NPAD % TW == 0
    NT_A = NPAD // TW  # full-table article proj tiles
    n_tw = [(t, min(TW, shard - t)) for t in range(0, shard, TW)]
    n_own_full = shard // TW  # full tiles inside own shard
    own_rem = shard % TW
    Gp = CH * CQp
    Gb = CH * CQb
    CBp, CBb = 4 * CQp, 4 * CQb

    nc = bacc.Bacc("TRN2", debug=False)

    # ---- I/O ----
    xaT = nc.dram_tensor("xaT", [DA_PAD, NPAD], BF16, kind="ExternalInput")
    xuT = nc.dram_tensor("xuT", [DU, shard], BF16, kind="ExternalInput")
    w_in_aT = nc.dram_tensor("w_in_aT", [DA_PAD, H], BF16, kind="ExternalInput")
    b_in_a = nc.dram_tensor("b_in_a", [H, 1], FP32, kind="ExternalInput")
    w_in_uT = nc.dram_tensor("w_in_uT", [DU, H], BF16, kind="ExternalInput")
    b_in_u = nc.dram_tensor("b_in_u", [H, 1], FP32, kind="ExternalInput")
    convw = {}
    for et in ("c1p", "c1b", "c2p"):
        convw[et] = (
            nc.dram_tensor(f"{et}_wlT", [H, H], BF16, kind="ExternalInput"),
            nc.dram_tensor(f"{et}_bl", [H, 1], FP32, kind="ExternalInput"),
            nc.dram_tensor(f"{et}_wrT", [H, H], BF16, kind="ExternalInput"),
        )
    w_outT = nc.dram_tensor("w_outT", [H, OUT], BF16, kind="ExternalInput")
    b_out = nc.dram_tensor("b_out", [OUT, 1], FP32, kind="ExternalInput")
    iota_in = nc.dram_tensor("iota", [128, 128], FP32, kind="ExternalInput")
    ident_in = nc.dram_tensor("ident", [128, 128], BF16, kind="ExternalInput")
    idx_p = nc.dram_tensor("idx_p", [128, NCHUNK * 4 * Gp * 8], I16, kind="ExternalInput")
    slot_p = nc.dram_tensor("slot_p", [128, NBLK * CBp], FP32, kind="ExternalInput")
    rval_p = nc.dram_tensor("rval_p", [128, NBLK * CBp], FP32, kind="ExternalInput")
    idx_b = nc.dram_tensor("idx_b", [128, NCHUNK * 4 * Gb * 8], I16, kind="ExternalInput")
    slot_b = nc.dram_tensor("slot_b", [128, NBLK * CBb], FP32, kind="ExternalInput")
    rval_b = nc.dram_tensor("rval_b", [128, NBLK * CBb], FP32, kind="ExternalInput")
    out_d = nc.dram_tensor("out", [OUT, shard], FP32, kind="ExternalOutput")

    # internal HBM
    atab = nc.dram_tensor("atab", [NPAD, 128], BF16)  # cols 0:64 = a (rotated)
    comb_shard = [
        nc.dram_tensor(f"comb_shard{k}", [RPA, 128], BF16) for k in range(NAG)
    ]
    comb_rm = nc.dram_tensor("comb_rm", [NPAD, 128], BF16, addr_space="Shared")
    groups = [list(range(n_cores))]

    from contextlib import ExitStack

    with tile.TileContext(nc) as tc, ExitStack() as _stack:
        cpool = _stack.enter_context(tc.tile_pool(name="const", bufs=1))
        iota_sb = cpool.tile([128, 128], FP32, tag="iota")
        ident_sb = cpool.tile([128, 128], BF16, tag="ident")
        nc.sync.dma_start(iota_sb[:], iota_in[:])
        nc.sync.dma_start(ident_sb[:], ident_in[:])

        def load_const(t, shape, dtype, tag):
            s = cpool.tile(shape, dtype, tag=tag)
            nc.sync.dma_start(s[:], t[:])
            return s

        w_in_aT_s = cpool.tile([128, len(KA), H], BF16, tag="w_in_aT")
        for ki, (k0, kn) in enumerate(KA):
            nc.sync.dma_start(w_in_aT_s[0:kn, ki, :], w_in_aT[k0 : k0 + kn, :])
        b_in_a_s = load_const(b_in_a, [H, 1], FP32, "b_in_a")
        w_in_uT_s = load_const(w_in_uT, [DU, H], BF16, "w_in_uT")
        b_in_u_s = load_const(b_in_u, [H, 1], FP32, "b_in_u")
        convw_s = {}
        for et in ("c1p", "c1b", "c2p"):
            wlT, bl, wrT = convw[et]
            convw_s[et] = (
                load_const(wlT, [H, H], BF16, f"{et}_wlT"),
                load_const(bl, [H, 1], FP32, f"{et}_bl"),
                load_const(wrT, [H, H], BF16, f"{et}_wrT"),
            )
        w_outT_s = load_const(w_outT, [H, OUT], BF16, "w_outT")
        b_out_s = load_const(b_out, [OUT, 1], FP32, "b_out")
        slot_p_s = load_const(slot_p, [128, NBLK * CBp], FP32, "slot_p")
        rval_p_s = load_const(rval_p, [128, NBLK * CBp], FP32, "rval_p")
        slot_b_s = load_const(slot_b, [128, NBLK * CBb], FP32, "slot_b")
        rval_b_s = load_const(rval_b, [128, NBLK * CBb], FP32, "rval_b")

        # resident feature-major node tables (own shard)
        uT_own = cpool.tile([H, shard], BF16, tag="uT_own")
        aT_own = cpool.tile([H, shard], BF16, tag="aT_own")

        # ------------------- stage 1: input projections -------------------
        def _inproj():
          with (
            tc.tile_pool(name="ip_ps", bufs=3, space="PSUM") as ip_ps,
            tc.tile_pool(name="tp_ps", bufs=2, space="PSUM") as tp_ps,
            tc.tile_pool(name="ip_sb", bufs=6) as ip_sb,
            tc.tile_pool(name="rel_sb", bufs=3) as rel_sb,
            tc.tile_pool(name="tp_sb", bufs=3) as tp_sb,
          ):
            # ---- user proj (own shard) ----
            for t0, tw in n_tw:
                xt = ip_sb.tile([DU, TW], BF16, tag="xu")
                nc.sync.dma_start(xt[:, 0:tw], xuT[:, t0 : t0 + tw])
                ps = ip_ps.tile([H, TW], FP32, tag="ipps")
                nc.tensor.matmul(ps[:, 0:tw], w_in_uT_s[:], xt[:, 0:tw])
                nc.scalar.activation(
                    uT_own[:, t0 : t0 + tw], ps[:, 0:tw], AF.Relu, bias=b_in_u_s[:]
                )
            # u -> comb_shard cols 0:64 (transposed, per block)
            for b in range(NBLK):
                tp = tp_ps.tile([128, H], BF16, tag="tpu")
                nc.tensor.transpose(
                    tp[:], uT_own[:, b * 128 : (b + 1) * 128], ident_sb[0:H, 0:H]
                )
                st = tp_sb.tile([128, H], BF16, tag="stu")
                nc.scalar.copy(st[:], tp[:])
                k, rb = b // BPA, b % BPA
                nc.sync.dma_start(
                    comb_shard[k][rb * 128 : (rb + 1) * 128, 0:H], st[:]
                )
            # ---- article proj (FULL table, rotated cols; own shard first) ----
            for t in range(NT_A):
                ps = ip_ps.tile([H, TW], FP32, tag="ipps")
                for ki, (k0, kn) in enumerate(KA):
                    xt = ip_sb.tile([128, TW], BF16, tag="xa")
                    nc.sync.dma_start(
                        xt[0:kn, :], xaT[k0 : k0 + kn, t * TW : (t + 1) * TW]
                    )
                    nc.tensor.matmul(
                        ps[:],
                        w_in_aT_s[0:kn, ki, :],
                        xt[0:kn, :],
                        start=(ki == 0),
                        stop=(ki == len(KA) - 1),
                    )
                rel = rel_sb.tile([H, TW], BF16, tag="rel")
                nc.scalar.activation(rel[:], ps[:], AF.Relu, bias=b_in_a_s[:])
                if t < n_own_full:
                    nc.vector.tensor_copy(aT_own[:, t * TW : (t + 1) * TW], rel[:])
                elif own_rem and t == n_own_full:
                    nc.vector.tensor_copy(
                        aT_own[:, n_own_full * TW : shard], rel[:, 0:own_rem]
                    )
                tp = tp_ps.tile([128, 4 * H], BF16, tag="tpa")
                for jj in range(4):
                    nc.tensor.transpose(
                        tp[:, jj * H : (jj + 1) * H],
                        rel[:, jj * 128 : (jj + 1) * 128],
                        ident_sb[0:H, 0:H],
                    )
                st = tp_sb.tile([128, 4 * H], BF16, tag="sta")
                nc.scalar.copy(st[:], tp[:])
                for jj in range(4):
                    r0 = t * TW + jj * 128
                    nc.sync.dma_start(
                        atab[r0 : r0 + 128, 0:H], st[:, jj * H : (jj + 1) * H]
                    )

        # ------------------- conv layers -------------------
        def conv_layer(pools, cpools, gtable, idx_dram, slot_s, rval_s, CQ, fused):
            """fused=False: c1b (users): u1 -> comb_shard cols 64:128 +
            chunked AG issue. fused=True: c1p + c2p + head -> out."""
            CB = 4 * CQ
            G = CH * CQ
            (msg_p, s_p, agg_ps, lin_ps, agg_sb, outb_p, idx_pool) = pools
            if fused:
                agg2_ps, hd_sb = cpools
            else:
                ctp_ps, ctp_sb = cpools
            if fused:
                wlT1, bl1, wrT1 = convw_s["c1p"]
                wlT2, bl2, wrT2 = convw_s["c2p"]
            else:
                wlT1, bl1, wrT1 = convw_s["c1b"]
            for ch in range(NCHUNK):
                it = idx_pool.tile([128, 4 * G * 8], I16, tag="idxs")
                nc.sync.dma_start(
                    it[:], idx_dram[:, ch * 4 * G * 8 : (ch + 1) * 4 * G * 8]
                )
                msg = msg_p.tile([128, 4, G, 128], BF16, tag="msg")
                if "gather" not in skip:
                    for q in range(4):
                        nc.gpsimd.dma_gather(
                            msg[:, q],
                            gtable[q * QN : (q + 1) * QN, :],
                            it[:, q * G * 8 : (q + 1) * G * 8],
                            G * 128,
                            G * 128,
                            128,
                            single_packet=False,
                        )
                elif ch == 0:
                    nc.vector.memset(msg[:], 0.0)
                for bb in range(CH):
                    b = ch * CH + bb
                    agg1t = agg_ps.tile([H, 128], FP32, tag="agg1")
                    agg1 = agg1t[:]
                    if fused:
                        agg2t = agg2_ps.tile([H, 128], FP32, tag="agg2")
                        agg2 = agg2t[:]
                    first, last = True, False
                    for q in range(4):
                        for tq in range(CQ):
                            col = b * CB + q * CQ + tq
                            last = q == 3 and tq == CQ - 1
                            S = s_p.tile([128, 128], BF16, tag="S")
                            nc.vector.tensor_scalar(
                                S[:],
                                iota_sb[:],
                                slot_s[:, col : col + 1],
                                rval_s[:, col : col + 1],
                                ALU.is_equal,
                                ALU.mult,
                            )
                            nc.tensor.matmul(
                                agg1,
                                msg[:, q, bb * CQ + tq, 0:H],
                                S[:],
                                start=first,
                                stop=last,
                            )
                            if fused:
                                nc.tensor.matmul(
                                    agg2,
                                    msg[:, q, bb * CQ + tq, H : 2 * H],
                                    S[:],
                                    start=first,
                                    stop=last,
                                )
                            first = False
                    aggs1 = agg_sb.tile([H, 128], BF16, tag="aggs1")
                    nc.scalar.copy(aggs1[:], agg1)
                    lin1 = lin_ps.tile([H, 128], FP32, tag="lin")
                    nc.tensor.matmul(lin1[:], wlT1[:], aggs1[:], start=True, stop=False)
                    xdst = aT_own if fused else uT_own
                    nc.tensor.matmul(
                        lin1[:],
                        wrT1[:],
                        xdst[:, b * 128 : (b + 1) * 128],
                        start=False,
                        stop=True,
                    )
                    o1 = outb_p.tile([H, 128], BF16, tag="o1")
                    nc.scalar.activation(o1[:], lin1[:], AF.Relu, bias=bl1[:])
                    if not fused:
                        # u1 block -> comb_shard cols 64:128 (transposed)
                        tp = ctp_ps.tile([128, H], BF16, tag="ctp")
                        nc.tensor.transpose(tp[:], o1[:], ident_sb[0:H, 0:H])
                        st = ctp_sb.tile([128, H], BF16, tag="cst")
                        nc.scalar.copy(st[:], tp[:])
                        k, rb = b // BPA, b % BPA
                        nc.sync.dma_start(
                            comb_shard[k][rb * 128 : (rb + 1) * 128, H : 2 * H],
                            st[:],
                        )
                    else:
                        aggs2 = agg_sb.tile([H, 128], BF16, tag="aggs2")
                        nc.scalar.copy(aggs2[:], agg2)
                        lin2 = lin_ps.tile([H, 128], FP32, tag="lin")
                        nc.tensor.matmul(
                            lin2[:], wlT2[:], aggs2[:], start=True, stop=False
                        )
                        nc.tensor.matmul(
                            lin2[:], wrT2[:], o1[:], start=False, stop=True
                        )
                        a2 = outb_p.tile([H, 128], BF16, tag="a2")
                        nc.vector.tensor_scalar_add(a2[:], lin2[:], bl2[:])
                        hp = lin_ps.tile([H, 128], FP32, tag="lin")
                        nc.tensor.matmul(hp[0:OUT, :], w_outT_s[:], a2[:])
                        ho = hd_sb.tile([OUT, 128], FP32, tag="hdo")
                        nc.vector.tensor_scalar_add(ho[:], hp[0:OUT, :], b_out_s[:])
                        nc.sync.dma_start(out_d[:, b * 128 : (b + 1) * 128], ho[:])
                if not fused and ch % AGC == AGC - 1 and "ag" not in skip:
                    k = ch // AGC
                    nc.gpsimd.collective_compute(
                        "AllGather",
                        ALU.bypass,
                        replica_groups=groups,
                        ins=[comb_shard[k][:]],
                        outs=[comb_rm[k * n_cores * RPA : (k + 1) * n_cores * RPA, :]],
                    )

        def _convs():
          with (
            tc.tile_pool(name="msg", bufs=2) as msg_p,
            tc.tile_pool(name="S", bufs=4) as s_p,
            tc.tile_pool(name="agg_ps", bufs=2, space="PSUM") as agg_ps,
            tc.tile_pool(name="lin_ps", bufs=3, space="PSUM") as lin_ps,
            tc.tile_pool(name="agg_sb", bufs=4) as agg_sb,
            tc.tile_pool(name="outb", bufs=4) as outb_p,
            tc.tile_pool(name="idxs", bufs=3) as idx_pool,
          ):
            pools = (msg_p, s_p, agg_ps, lin_ps, agg_sb, outb_p, idx_pool)
            # users conv first (produces u1, issues chunked AllGathers)
            with (
                tc.tile_pool(name="ctp_ps", bufs=2, space="PSUM") as ctp_ps,
                tc.tile_pool(name="ctp_sb", bufs=3) as ctp_sb,
            ):
                conv_layer(pools, (ctp_ps, ctp_sb), atab, idx_b,
                           slot_b_s, rval_b_s, CQb, False)
            # fused c1p + c2p + head over article dsts
            with (
                tc.tile_pool(name="agg2_ps", bufs=2, space="PSUM") as agg2_ps,
                tc.tile_pool(name="hd_sb", bufs=3) as hd_sb,
            ):
                conv_layer(pools, (agg2_ps, hd_sb), comb_rm, idx_p,
                           slot_p_s, rval_p_s, CQp, True)

        for _rep in range(reps):
            _inproj()
            if "convs" not in skip:
                _convs()

    nc.compile()
    return nc


# ----------------------------------------------------------------------------
# Entry point
# ----------------------------------------------------------------------------

_CACHE = {}


def build_in_maps(inputs, cfg, CQp, per_core_p, CQb, per_core_b):
    N, DA, DU, H = cfg["N"], cfg["DA"], cfg["DU"], cfg["H"]
    n_cores, shard = cfg["n_cores"], cfg["shard"]
    NPAD = n_cores * shard
    DA_PAD = ((DA + 15) // 16) * 16
    xa = np.asarray(inputs["x_article"], np.float32)
    xu = np.asarray(inputs["x_user"], np.float32)

    shared = dict(
        w_in_aT=np.concatenate(
            [_lin_bf16(inputs["w_in_a"]), np.zeros((DA_PAD - DA, H), BF16_NP)], 0
        ),
        b_in_a=_bias_col(inputs["b_in_a"]),
        w_in_uT=_lin_bf16(inputs["w_in_u"]),
        b_in_u=_bias_col(inputs["b_in_u"]),
        w_outT=_lin_bf16(inputs["w_out"]),
        b_out=_bias_col(inputs["b_out"]),
        iota=np.tile(np.arange(128, dtype=np.float32), (128, 1)),
        ident=np.eye(128, dtype=BF16_NP),
    )
    for et in ("c1p", "c1b", "c2p"):
        shared[f"{et}_wlT"] = _lin_bf16(inputs[f"{et}_wl"])
        shared[f"{et}_bl"] = _bias_col(inputs[f"{et}_bl"])
        shared[f"{et}_wrT"] = _lin_bf16(inputs[f"{et}_wr"])

    # full article feature table, feature-major, padded
    xaT_nat = np.zeros((DA_PAD, NPAD), BF16_NP)
    xaT_nat[:DA, :N] = xa.T.astype(BF16_NP)

    in_maps = []
    for c in range(n_cores):
        c0, c1 = c * shard, min((c + 1) * shard, N)
        xuT_c = np.zeros((DU, shard), BF16_NP)
        xuT_c[:, : c1 - c0] = xu[c0:c1].T.astype(BF16_NP)
        m = dict(shared)
        m["xaT"] = np.roll(xaT_nat, -c * shard, axis=1)
        m["xuT"] = xuT_c
        m["idx_p"] = per_core_p[c]["idx_w"]
        m["slot_p"] = per_core_p[c]["slot_w"]
        m["rval_p"] = per_core_p[c]["rval_w"]
        m["idx_b"] = per_core_b[c]["idx_w"]
        m["slot_b"] = per_core_b[c]["slot_w"]
        m["rval_b"] = per_core_b[c]["rval_w"]
        in_maps.append(m)
    return in_maps


def _prep_all(inputs, cfg):
    n_cores, shard = cfg["n_cores"], cfg["shard"]
    NPAD = n_cores * shard
    # posts: user -> article. src users, table row = AG-chunk layout.
    src_p = np.asarray(inputs["ei_posts"][0], np.int64)
    dst_p = np.asarray(inputs["ei_posts"][1], np.int64)
    trow_p = user_trow(src_p, cfg)
    CQp, per_core_p = prep_edges([trow_p] * n_cores, dst_p, cfg)
    # posted_by: article -> user. src articles, per-core rotated rows.
    src_b = np.asarray(inputs["ei_pb"][0], np.int64)
    dst_b = np.asarray(inputs["ei_pb"][1], np.int64)
    trows_b = [(src_b - c * shard) % NPAD for c in range(n_cores)]
    CQb, per_core_b = prep_edges(trows_b, dst_b, cfg)
    return CQp, per_core_p, CQb, per_core_b


def _run(inputs, cfg, trace=False, reps=1):
    N, n_cores, shard = cfg["N"], cfg["n_cores"], cfg["shard"]

    CQp, per_core_p, CQb, per_core_b = _prep_all(inputs, cfg)

    key = (tuple(sorted(cfg.items())), CQp, CQb, reps)
    if key not in _CACHE:
        _CACHE[key] = build_program(cfg, CQp, CQb, reps)
    nc = _CACHE[key]

    in_maps = build_in_maps(inputs, cfg, CQp, per_core_p, CQb, per_core_b)

    res = run_bass_kernel_spmd(nc, in_maps, list(range(n_cores)), trace=trace)
    outs = [res.results[c]["out"] for c in range(n_cores)]  # [2, shard] each
    full = np.concatenate(outs, axis=1)[:, :N].T.astype(np.float32)
    return np.ascontiguousarray(full), res


def kernel(**inputs):
    out, _ = _run(inputs, full_cfg(), trace=False)
    return out


# revision 22
# speedup vs baseline: 2.7106x; 2.7106x over previous
"""Trainium2 Bass kernel for hetero GNN (2x SAGEConv layers + in/out proj).

Full inputs in, full output out. Design (v2):
- dst-node sharding across 8 cores (shard=12544 per core).
- Article input projection REPLICATED on every core (per-core column-rotated
  xaT input so each core's own shard lands at columns [0, shard)) -> local
  full article table `atab`, no input AllGathers.
- User projection sharded; u goes into cols 0:64 of a combined per-AG-chunk
  shard buffer; conv c1b writes u1 into cols 64:128. Seven chunked
  AllGathers (issued inside the c1b block loop) build the combined
  [u | u1] table `comb_rm`, hidden under c1b compute.
- conv c1p + c2p FUSED: one gather pass reads 256B rows carrying both u and
  u1, S (one-hot scatter) matrices built once and used by both PSUM chains.
- Gathers chunked: CH blocks x 4 src-quadrants per dma_gather call.
"""

import math

import numpy as np

import concourse.bacc as bacc
import concourse.bass as bass
import concourse.mybir as mybir
from concourse import tile
from concourse.bass_utils import run_bass_kernel_spmd

FP32 = mybir.dt.float32
BF16 = mybir.dt.bfloat16
I16 = mybir.dt.int16
AF = mybir.ActivationFunctionType
ALU = mybir.AluOpType

BF16_NP = mybir.dt.np(BF16)


def full_cfg():
    return dict(
        N=100000,
        E=1600000,
        DA=300,
        DU=64,
        H=64,
        OUT=2,
        n_cores=8,
        shard=12544,  # 98 * 128 per-core dst shard
        cq_min=5,
        CH=7,   # dst blocks per gather chunk (must divide NBLK)
        AGC=2,  # gather chunks per AllGather chunk
    )


# ----------------------------------------------------------------------------
# Host-side edge preprocessing
# ----------------------------------------------------------------------------


def user_trow(u, cfg):
    """Table row of user u in the chunk-AllGathered combined table."""
    shard, n_cores = cfg["shard"], cfg["n_cores"]
    RPA = cfg["CH"] * cfg["AGC"] * 128  # rows per AG chunk per core
    cu = u // shard
    r = u % shard
    return (r // RPA) * (n_cores * RPA) + cu * RPA + (r % RPA)


def prep_edges(src_t_per_core, dst, cfg):
    """Bucket edges by (dst gather-chunk, src quadrant, block-in-chunk).

    src_t_per_core: [n_cores, E] table row of the source node as seen by
    each core (articles: per-core rotated; users: same for all cores).
    dst: [E] global dst id (dst-sharded by range).

    Returns (CQ, per_core list of dicts idx_w/slot_w/rval_w).
    """
    N, shard, n_cores = cfg["N"], cfg["shard"], cfg["n_cores"]
    CH = cfg["CH"]
    NPAD = n_cores * shard
    QN = NPAD // 4
    assert QN < 32768, QN
    NBLK = shard // 128
    NCHUNK = NBLK // CH
    assert NCHUNK * CH == NBLK

    dst = np.asarray(dst, dtype=np.int64)
    E = len(dst)
    deg = np.bincount(dst, minlength=N).astype(np.float64)
    recip = (1.0 / np.maximum(deg, 1.0)).astype(np.float32)

    core = dst // shard
    bl = (dst % shard) >> 7
    slot_val = (dst & 127).astype(np.float32)
    rval_val = recip[dst]
    ch = bl // CH
    bb = bl % CH

    # quadrant depends on per-core src table row; compute per core
    per_core = []
    CQ = cfg["cq_min"]
    # first pass: find global CQ
    cell_list = []
    for c in range(n_cores):
        m = core == c
        st = src_t_per_core[c][m]
        q = st // QN
        cell = (ch[m] * 4 + q) * CH + bb[m]
        cell_list.append((m, st, q, cell))
        cnts = np.bincount(cell, minlength=NCHUNK * 4 * CH)
        if len(cnts):
            CQ = max(CQ, int(math.ceil(cnts.max() / 128)))
    CB = 4 * CQ
    G = CH * CQ  # 128-edge groups per (chunk, quadrant) gather call

    for c in range(n_cores):
        m, st, q, cell = cell_list[c]
        order = np.argsort(cell, kind="stable")
        cell_s = cell[order]
        n_cells = NCHUNK * 4 * CH
        starts = np.searchsorted(cell_s, np.arange(n_cells))
        j = np.arange(len(cell_s)) - starts[cell_s]  # position within cell
        st_s = st[order]
        q_s = q[order]
        ch_s = ch[m][order]
        bb_s = bb[m][order]
        bl_s = bl[m][order]
        loc_idx = (st_s % QN).astype(np.int16)
        sv = slot_val[m][order]
        rv = rval_val[m][order]

        # gather idx array: per call (ch, q), position J = bb*CQ*128 + j,
        # wrapped into 16 partitions, replicated 8x down the partition dim.
        idx_w = np.zeros((128, NCHUNK * 4 * G * 8), dtype=np.int16)
        J = bb_s * (CQ * 128) + j
        col = (ch_s * 4 + q_s) * (G * 8) + J // 16
        row = (J % 16).astype(np.int64)
        for g in range(8):
            idx_w[row + 16 * g, col] = loc_idx
        # slot / recip one-hot source arrays: [128, NBLK*CB]
        slot_w = np.full((128, NBLK * CB), 999.0, dtype=np.float32)
        rval_w = np.zeros((128, NBLK * CB), dtype=np.float32)
        colS = bl_s * CB + q_s * CQ + j // 128
        rowS = j % 128
        slot_w[rowS, colS] = sv
        rval_w[rowS, colS] = rv
        per_core.append(dict(idx_w=idx_w, slot_w=slot_w, rval_w=rval_w))
    return CQ, per_core


def _lin_bf16(w):
    """[out,in] fp32 -> lhsT layout [in,out] bf16."""
    return np.ascontiguousarray(np.asarray(w).T).astype(BF16_NP)


def _bias_col(b):
    return np.asarray(b, np.float32).reshape(-1, 1)


# ----------------------------------------------------------------------------
# Device program
# ----------------------------------------------------------------------------


def build_program(cfg, CQp, CQb, reps=1, skip=()):
    N, DA, DU, H, OUT = cfg["N"], cfg["DA"], cfg["DU"], cfg["H"], cfg["OUT"]
    n_cores, shard = cfg["n_cores"], cfg["shard"]
    CH, AGC = cfg["CH"], cfg["AGC"]
    NPAD = n_cores * shard
    QN = NPAD // 4
    NBLK = shard // 128
    NCHUNK = NBLK // CH
    NAG = NCHUNK // AGC
    assert NAG * AGC == NCHUNK
    RPA = CH * AGC * 128  # rows per AG chunk (per core)
    BPA = CH * AGC        # blocks per AG chunk
    DA_PAD = ((DA + 15) // 16) * 16  # 304
    KA = [(k, min(128, DA_PAD - k)) for k in range(0, DA_PAD, 128)]
    TW = 512
    assert NPAD % TW == 0
    NT_A = NPAD // TW  # full-table article proj tiles
    n_tw = [(t, min(TW, shard - t)) for t in range(0, shard, TW)]
    n_own_full = shard // TW  # full tiles inside own shard
    own_rem = shard % TW
    Gp = CH * CQp
    Gb = CH * CQb
    CBp, CBb = 4 * CQp, 4 * CQb

    nc = bacc.Bacc("TRN2", debug=False)

    # ---- I/O ----
    xaT = nc.dram_tensor("xaT", [DA_PAD, NPAD], BF16, kind="ExternalInput")
    xuT = nc.dram_tensor("xuT", [DU, shard], BF16, kind="ExternalInput")
    w_in_aT = nc.dram_tensor("w_in_aT", [DA_PAD, H], BF16, kind="ExternalInput")
    b_in_a = nc.dram_tensor("b_in_a", [H, 1], FP32, kind="ExternalInput")
    w_in_uT = nc.dram_tensor("w_in_uT", [DU, H], BF16, kind="ExternalInput")
    b_in_u = nc.dram_tensor("b_in_u", [H, 1], FP32, kind="ExternalInput")
    convw = {}
    for et in ("c1p", "c1b", "c2p"):
        convw[et] = (
            nc.dram_tensor(f"{et}_wlT", [H, H], BF16, kind="ExternalInput"),
            nc.dram_tensor(f"{et}_bl", [H, 1], FP32, kind="ExternalInput"),
            nc.dram_tensor(f"{et}_wrT", [H, H], BF16, kind="ExternalInput"),
        )
    w_outT = nc.dram_tensor("w_outT", [H, OUT], BF16, kind="ExternalInput")
    b_out = nc.dram_tensor("b_out", [OUT, 1], FP32, kind="ExternalInput")
    iota_in = nc.dram_tensor("iota", [128, 128], FP32, kind="ExternalInput")
    ident_in = nc.dram_tensor("ident", [128, 128], BF16, kind="ExternalInput")
    idx_p = nc.dram_tensor("idx_p", [128, NCHUNK * 4 * Gp * 8], I16, kind="ExternalInput")
    slot_p = nc.dram_tensor("slot_p", [128, NBLK * CBp], FP32, kind="ExternalInput")
    rval_p = nc.dram_tensor("rval_p", [128, NBLK * CBp], FP32, kind="ExternalInput")
    idx_b = nc.dram_tensor("idx_b", [128, NCHUNK * 4 * Gb * 8], I16, kind="ExternalInput")
    slot_b = nc.dram_tensor("slot_b", [128, NBLK * CBb], FP32, kind="ExternalInput")
    rval_b = nc.dram_tensor("rval_b", [128, NBLK * CBb], FP32, kind="ExternalInput")
    out_d = nc.dram_tensor("out", [OUT, shard], FP32, kind="ExternalOutput")

    # internal HBM
    atab = nc.dram_tensor("atab", [NPAD, 128], BF16)  # cols 0:64 = a (rotated)
    comb_shard = [
        nc.dram_tensor(f"comb_shard{k}", [RPA, 128], BF16) for k in range(NAG)
    ]
    comb_rm = nc.dram_tensor("comb_rm", [NPAD, 128], BF16, addr_space="Shared")
    groups = [list(range(n_cores))]

    from contextlib import ExitStack

    with tile.TileContext(nc) as tc, ExitStack() as _stack:
        cpool = _stack.enter_context(tc.tile_pool(name="const", bufs=1))
        iota_sb = cpool.tile([128, 128], FP32, tag="iota")
        ident_sb = cpool.tile([128, 128], BF16, tag="ident")
        nc.sync.dma_start(iota_sb[:], iota_in[:])
        nc.sync.dma_start(ident_sb[:], ident_in[:])

        def load_const(t, shape, dtype, tag):
            s = cpool.tile(shape, dtype, tag=tag)
            nc.sync.dma_start(s[:], t[:])
            return s

        w_in_aT_s = cpool.tile([128, len(KA), H], BF16, tag="w_in_aT")
        for ki, (k0, kn) in enumerate(KA):
            nc.sync.dma_start(w_in_aT_s[0:kn, ki, :], w_in_aT[k0 : k0 + kn, :])
        b_in_a_s = load_const(b_in_a, [H, 1], FP32, "b_in_a")
        w_in_uT_s = load_const(w_in_uT, [DU, H], BF16, "w_in_uT")
        b_in_u_s = load_const(b_in_u, [H, 1], FP32, "b_in_u")
        convw_s = {}
        for et in ("c1p", "c1b", "c2p"):
            wlT, bl, wrT = convw[et]
            convw_s[et] = (
                load_const(wlT, [H, H], BF16, f"{et}_wlT"),
                load_const(bl, [H, 1], FP32, f"{et}_bl"),
                load_const(wrT, [H, H], BF16, f"{et}_wrT"),
            )
        w_outT_s = load_const(w_outT, [H, OUT], BF16, "w_outT")
        b_out_s = load_const(b_out, [OUT, 1], FP32, "b_out")
        slot_p_s = load_const(slot_p, [128, NBLK * CBp], FP32, "slot_p")
        rval_p_s = load_const(rval_p, [128, NBLK * CBp], FP32, "rval_p")
        slot_b_s = load_const(slot_b, [128, NBLK * CBb], FP32, "slot_b")
        rval_b_s = load_const(rval_b, [128, NBLK * CBb], FP32, "rval_b")

        # resident feature-major node tables (own shard)
        uT_own = cpool.tile([H, shard], BF16, tag="uT_own")
        aT_own = cpool.tile([H, shard], BF16, tag="aT_own")

        # ------------------- stage 1: input projections -------------------
        def _inproj():
          with (
            tc.tile_pool(name="ip_ps", bufs=3, space="PSUM") as ip_ps,
            tc.tile_pool(name="tp_ps", bufs=2, space="PSUM") as tp_ps,
            tc.tile_pool(name="ip_sb", bufs=6) as ip_sb,
            tc.tile_pool(name="rel_sb", bufs=3) as rel_sb,
            tc.tile_pool(name="tp_sb", bufs=3) as tp_sb,
          ):
            # ---- user proj (own shard) ----
            for t0, tw in n_tw:
                xt = ip_sb.tile([DU, TW], BF16, tag="xu")
                nc.sync.dma_start(xt[:, 0:tw], xuT[:, t0 : t0 + tw])
                ps = ip_ps.tile([H, TW], FP32, tag="ipps")
                nc.tensor.matmul(ps[:, 0:tw], w_in_uT_s[:], xt[:, 0:tw])
                nc.scalar.activation(
                    uT_own[:, t0 : t0 + tw], ps[:, 0:tw], AF.Relu, bias=b_in_u_s[:]
                )
            # u -> comb_shard cols 0:64 (transposed, per block)
            for b in range(NBLK):
                tp = tp_ps.tile([128, H], BF16, tag="tpu")
                nc.tensor.transpose(
                    tp[:], uT_own[:, b * 128 : (b + 1) * 128], ident_sb[0:H, 0:H]
                )
                st = tp_sb.tile([128, H], BF16, tag="stu")
                nc.scalar.copy(st[:], tp[:])
                k, rb = b // BPA, b % BPA
                nc.sync.dma_start(
                    comb_shard[k][rb * 128 : (rb + 1) * 128, 0:H], st[:]
                )
            # ---- article proj (FULL table, rotated cols; own shard first) ----
            for t in range(NT_A):
                ps = ip_ps.tile([H, TW], FP32, tag="ipps")
                for ki, (k0, kn) in enumerate(KA):
                    xt = ip_sb.tile([128, TW], BF16, tag="xa")
                    nc.sync.dma_start(
                        xt[0:kn, :], xaT[k0 : k0 + kn, t * TW : (t + 1) * TW]
                    )
                    nc.tensor.matmul(
                        ps[:],
                        w_in_aT_s[0:kn, ki, :],
                        xt[0:kn, :],
                        start=(ki == 0),
                        stop=(ki == len(KA) - 1),
                    )
                rel = rel_sb.tile([H, TW], BF16, tag="rel")
                nc.scalar.activation(rel[:], ps[:], AF.Relu, bias=b_in_a_s[:])
                if t < n_own_full:
                    nc.vector.tensor_copy(aT_own[:, t * TW : (t + 1) * TW], rel[:])
                elif own_rem and t == n_own_full:
                    nc.vector.tensor_copy(
                        aT_own[:, n_own_full * TW : shard], rel[:, 0:own_rem]
                    )
                tp = tp_ps.tile([128, 4 * H], BF16, tag="tpa")
                for jj in range(4):
                    nc.tensor.transpose(
                        tp[:, jj * H : (jj + 1) * H],
                        rel[:, jj * 128 : (jj + 1) * 128],
                        ident_sb[0:H, 0:H],
                    )
                st = tp_sb.tile([128, 4 * H], BF16, tag="sta")
                nc.scalar.copy(st[:], tp[:])
                for jj in range(4):
                    r0 = t * TW + jj * 128
                    nc.sync.dma_start(
                        atab[r0 : r0 + 128, 0:H], st[:, jj * H : (jj + 1) * H]
                    )

        # ------------------- conv layers -------------------
        def conv_layer(pools, cpools, gtable, idx_dram, slot_s, rval_s, CQ, fused):
            """fused=False: c1b (users): u1 -> comb_shard cols 64:128 +
            chunked AG issue. fused=True: c1p + c2p + head -> out."""
            CB = 4 * CQ
            G = CH * CQ
            (msg_p, s_p, agg_ps, lin_ps, agg_sb, outb_p, idx_pool) = pools
            if fused:
                agg2_ps, hd_sb = cpools
            else:
                ctp_ps, ctp_sb = cpools
            if fused:
                wlT1, bl1, wrT1 = convw_s["c1p"]
                wlT2, bl2, wrT2 = convw_s["c2p"]
            else:
                wlT1, bl1, wrT1 = convw_s["c1b"]
            for ch in range(NCHUNK):
                it = idx_pool.tile([128, 4 * G * 8], I16, tag="idxs")
                nc.sync.dma_start(
                    it[:], idx_dram[:, ch * 4 * G * 8 : (ch + 1) * 4 * G * 8]
                )
                msg = msg_p.tile([128, 4, G, 128], BF16, tag="msg")
                if "gather" not in skip:
                    # <=1024 idx (64 descs/lane) per call: single_packet mode
                    GS = max(d for d in range(1, 9) if G % d == 0)
                    for q in range(4):
                        for s in range(G // GS):
                            nc.gpsimd.dma_gather(
                                msg[:, q, s * GS : (s + 1) * GS, :],
                                gtable[q * QN : (q + 1) * QN, :],
                                it[:, (q * G + s * GS) * 8 : (q * G + (s + 1) * GS) * 8],
                                GS * 128,
                                GS * 128,
                                128,
                            )
                elif ch == 0:
                    nc.vector.memset(msg[:], 0.0)
                for bb in range(CH):
                    b = ch * CH + bb
                    agg1t = agg_ps.tile([H, 128], FP32, tag="agg1")
                    agg1 = agg1t[:]
                    if fused:
                        agg2t = agg2_ps.tile([H, 128], FP32, tag="agg2")
                        agg2 = agg2t[:]
                    first, last = True, False
                    for q in range(4):
                        for tq in range(CQ):
                            col = b * CB + q * CQ + tq
                            last = q == 3 and tq == CQ - 1
                            S = s_p.tile([128, 128], BF16, tag="S")
                            nc.vector.tensor_scalar(
                                S[:],
                                iota_sb[:],
                                slot_s[:, col : col + 1],
                                rval_s[:, col : col + 1],
                                ALU.is_equal,
                                ALU.mult,
                            )
                            nc.tensor.matmul(
                                agg1,
                                msg[:, q, bb * CQ + tq, 0:H],
                                S[:],
                                start=first,
                                stop=last,
                            )
                            if fused:
                                nc.tensor.matmul(
                                    agg2,
                                    msg[:, q, bb * CQ + tq, H : 2 * H],
                                    S[:],
                                    start=first,
                                    stop=last,
                                )
                            first = False
                    aggs1 = agg_sb.tile([H, 128], BF16, tag="aggs1")
                    nc.scalar.copy(aggs1[:], agg1)
                    lin1 = lin_ps.tile([H, 128], FP32, tag="lin")
                    nc.tensor.matmul(lin1[:], wlT1[:], aggs1[:], start=True, stop=False)
                    xdst = aT_own if fused else uT_own
                    nc.tensor.matmul(
                        lin1[:],
                        wrT1[:],
                        xdst[:, b * 128 : (b + 1) * 128],
                        start=False,
                        stop=True,
                    )
                    o1 = outb_p.tile([H, 128], BF16, tag="o1")
                    nc.scalar.activation(o1[:], lin1[:], AF.Relu, bias=bl1[:])
                    if not fused:
                        # u1 block -> comb_shard cols 64:128 (transposed)
                        tp = ctp_ps.tile([128, H], BF16, tag="ctp")
                        nc.tensor.transpose(tp[:], o1[:], ident_sb[0:H, 0:H])
                        st = ctp_sb.tile([128, H], BF16, tag="cst")
                        nc.scalar.copy(st[:], tp[:])
                        k, rb = b // BPA, b % BPA
                        nc.sync.dma_start(
                            comb_shard[k][rb * 128 : (rb + 1) * 128, H : 2 * H],
                            st[:],
                        )
                    else:
                        aggs2 = agg_sb.tile([H, 128], BF16, tag="aggs2")
                        nc.scalar.copy(aggs2[:], agg2)
                        lin2 = lin_ps.tile([H, 128], FP32, tag="lin")
                        nc.tensor.matmul(
                            lin2[:], wlT2[:], aggs2[:], start=True, stop=False
                        )
                        nc.tensor.matmul(
                            lin2[:], wrT2[:], o1[:], start=False, stop=True
                        )
                        a2 = outb_p.tile([H, 128], BF16, tag="a2")
                        nc.vector.tensor_scalar_add(a2[:], lin2[:], bl2[:])
                        hp = lin_ps.tile([H, 128], FP32, tag="lin")
                        nc.tensor.matmul(hp[0:OUT, :], w_outT_s[:], a2[:])
                        ho = hd_sb.tile([OUT, 128], FP32, tag="hdo")
                        nc.vector.tensor_scalar_add(ho[:], hp[0:OUT, :], b_out_s[:])
                        nc.sync.dma_start(out_d[:, b * 128 : (b + 1) * 128], ho[:])
                if not fused and ch % AGC == AGC - 1 and "ag" not in skip:
                    k = ch // AGC
                    nc.gpsimd.collective_compute(
                        "AllGather",
                        ALU.bypass,
                        replica_groups=groups,
                        ins=[comb_shard[k][:]],
                        outs=[comb_rm[k * n_cores * RPA : (k + 1) * n_cores * RPA, :]],
                    )

        def _convs():
          with (
            tc.tile_pool(name="msg", bufs=2) as msg_p,
            tc.tile_pool(name="S", bufs=4) as s_p,
            tc.tile_pool(name="agg_ps", bufs=2, space="PSUM") as agg_ps,
            tc.tile_pool(name="lin_ps", bufs=3, space="PSUM") as lin_ps,
            tc.tile_pool(name="agg_sb", bufs=4) as agg_sb,
            tc.tile_pool(name="outb", bufs=4) as outb_p,
            tc.tile_pool(name="idxs", bufs=3) as idx_pool,
          ):
            pools = (msg_p, s_p, agg_ps, lin_ps, agg_sb, outb_p, idx_pool)
            # users conv first (produces u1, issues chunked AllGathers)
            with (
                tc.tile_pool(name="ctp_ps", bufs=2, space="PSUM") as ctp_ps,
                tc.tile_pool(name="ctp_sb", bufs=3) as ctp_sb,
            ):
                conv_layer(pools, (ctp_ps, ctp_sb), atab, idx_b,
                           slot_b_s, rval_b_s, CQb, False)
            # fused c1p + c2p + head over article dsts
            with (
                tc.tile_pool(name="agg2_ps", bufs=2, space="PSUM") as agg2_ps,
                tc.tile_pool(name="hd_sb", bufs=3) as hd_sb,
            ):
                conv_layer(pools, (agg2_ps, hd_sb), comb_rm, idx_p,
                           slot_p_s, rval_p_s, CQp, True)

        for _rep in range(reps):
            _inproj()
            if "convs" not in skip:
                _convs()

    nc.compile()
    return nc


# ----------------------------------------------------------------------------
# Entry point
# ----------------------------------------------------------------------------

_CACHE = {}


def build_in_maps(inputs, cfg, CQp, per_core_p, CQb, per_core_b):
    N, DA, DU, H = cfg["N"], cfg["DA"], cfg["DU"], cfg["H"]
    n_cores, shard = cfg["n_cores"], cfg["shard"]
    NPAD = n_cores * shard
    DA_PAD = ((DA + 15) // 16) * 16
    xa = np.asarray(inputs["x_article"], np.float32)
    xu = np.asarray(inputs["x_user"], np.float32)

    shared = dict(
        w_in_aT=np.concatenate(
            [_lin_bf16(inputs["w_in_a"]), np.zeros((DA_PAD - DA, H), BF16_NP)], 0
        ),
        b_in_a=_bias_col(inputs["b_in_a"]),
        w_in_uT=_lin_bf16(inputs["w_in_u"]),
        b_in_u=_bias_col(inputs["b_in_u"]),
        w_outT=_lin_bf16(inputs["w_out"]),
        b_out=_bias_col(inputs["b_out"]),
        iota=np.tile(np.arange(128, dtype=np.float32), (128, 1)),
        ident=np.eye(128, dtype=BF16_NP),
    )
    for et in ("c1p", "c1b", "c2p"):
        shared[f"{et}_wlT"] = _lin_bf16(inputs[f"{et}_wl"])
        shared[f"{et}_bl"] = _bias_col(inputs[f"{et}_bl"])
        shared[f"{et}_wrT"] = _lin_bf16(inputs[f"{et}_wr"])

    # full article feature table, feature-major, padded
    xaT_nat = np.zeros((DA_PAD, NPAD), BF16_NP)
    xaT_nat[:DA, :N] = xa.T.astype(BF16_NP)

    in_maps = []
    for c in range(n_cores):
        c0, c1 = c * shard, min((c + 1) * shard, N)
        xuT_c = np.zeros((DU, shard), BF16_NP)
        xuT_c[:, : c1 - c0] = xu[c0:c1].T.astype(BF16_NP)
        m = dict(shared)
        m["xaT"] = np.roll(xaT_nat, -c * shard, axis=1)
        m["xuT"] = xuT_c
        m["idx_p"] = per_core_p[c]["idx_w"]
        m["slot_p"] = per_core_p[c]["slot_w"]
        m["rval_p"] = per_core_p[c]["rval_w"]
        m["idx_b"] = per_core_b[c]["idx_w"]
        m["slot_b"] = per_core_b[c]["slot_w"]
        m["rval_b"] = per_core_b[c]["rval_w"]
        in_maps.append(m)
    return in_maps


def _prep_all(inputs, cfg):
    n_cores, shard = cfg["n_cores"], cfg["shard"]
    NPAD = n_cores * shard
    # posts: user -> article. src users, table row = AG-chunk layout.
    src_p = np.asarray(inputs["ei_posts"][0], np.int64)
    dst_p = np.asarray(inputs["ei_posts"][1], np.int64)
    trow_p = user_trow(src_p, cfg)
    CQp, per_core_p = prep_edges([trow_p] * n_cores, dst_p, cfg)
    # posted_by: article -> user. src articles, per-core rotated rows.
    src_b = np.asarray(inputs["ei_pb"][0], np.int64)
    dst_b = np.asarray(inputs["ei_pb"][1], np.int64)
    trows_b = [(src_b - c * shard) % NPAD for c in range(n_cores)]
    CQb, per_core_b = prep_edges(trows_b, dst_b, cfg)
    return CQp, per_core_p, CQb, per_core_b


def _run(inputs, cfg, trace=False, reps=1):
    N, n_cores, shard = cfg["N"], cfg["n_cores"], cfg["shard"]

    CQp, per_core_p, CQb, per_core_b = _prep_all(inputs, cfg)

    key = (tuple(sorted(cfg.items())), CQp, CQb, reps)
    if key not in _CACHE:
        _CACHE[key] = build_program(cfg, CQp, CQb, reps)
    nc = _CACHE[key]

    in_maps = build_in_maps(inputs, cfg, CQp, per_core_p, CQb, per_core_b)

    res = run_bass_kernel_spmd(nc, in_maps, list(range(n_cores)), trace=trace)
    outs = [res.results[c]["out"] for c in range(n_cores)]  # [2, shard] each
    full = np.concatenate(outs, axis=1)[:, :N].T.astype(np.float32)
    return np.ascontiguousarray(full), res


def kernel(**inputs):
    out, _ = _run(inputs, full_cfg(), trace=False)
    return out


# revision 41
# speedup vs baseline: 9.4742x; 3.4953x over previous
"""Trainium2 Bass kernel for hetero GNN (2x SAGEConv layers + in/out proj).

Full inputs in, full output out. Design (v2):
- dst-node sharding across 8 cores (shard=12544 per core).
- Article input projection REPLICATED on every core (per-core column-rotated
  xaT input so each core's own shard lands at columns [0, shard)) -> local
  full article table `atab`, no input AllGathers.
- User projection sharded; u goes into cols 0:64 of a combined per-AG-chunk
  shard buffer; conv c1b writes u1 into cols 64:128. Seven chunked
  AllGathers (issued inside the c1b block loop) build the combined
  [u | u1] table `comb_rm`, hidden under c1b compute.
- conv c1p + c2p FUSED: one gather pass reads 256B rows carrying both u and
  u1, S (one-hot scatter) matrices built once and used by both PSUM chains.
- Gathers chunked: CH blocks x 4 src-quadrants per dma_gather call.
"""

import math

import numpy as np

import concourse.bacc as bacc
import concourse.bass as bass
import concourse.mybir as mybir
from concourse import tile
from concourse.bass_utils import run_bass_kernel_spmd

FP32 = mybir.dt.float32
BF16 = mybir.dt.bfloat16
I16 = mybir.dt.int16
AF = mybir.ActivationFunctionType
ALU = mybir.AluOpType

BF16_NP = mybir.dt.np(BF16)


def full_cfg():
    return dict(
        N=100000,
        E=1600000,
        DA=300,
        DU=64,
        H=64,
        OUT=2,
        n_cores=8,
        shard=12544,  # 98 * 128 per-core dst shard
        cq_min=5,
        CH=7,   # dst blocks per gather chunk (must divide NBLK)
        AGC=2,  # gather chunks per AllGather chunk
    )


# ----------------------------------------------------------------------------
# Host-side edge preprocessing
# ----------------------------------------------------------------------------


def user_trow(u, cfg):
    """Table row of user u in the chunk-AllGathered combined table."""
    shard, n_cores = cfg["shard"], cfg["n_cores"]
    RPA = cfg["CH"] * cfg["AGC"] * 128  # rows per AG chunk per core
    cu = u // shard
    r = u % shard
    return (r // RPA) * (n_cores * RPA) + cu * RPA + (r % RPA)


def prep_edges(src_t_per_core, dst, cfg):
    """Bucket edges by (dst gather-chunk, src quadrant, block-in-chunk).

    src_t_per_core: [n_cores, E] table row of the source node as seen by
    each core (articles: per-core rotated; users: same for all cores).
    dst: [E] global dst id (dst-sharded by range).

    Returns (CQ, per_core list of dicts idx_w/slot_w/rval_w).
    """
    N, shard, n_cores = cfg["N"], cfg["shard"], cfg["n_cores"]
    CH = cfg["CH"]
    NPAD = n_cores * shard
    QN = NPAD // 4
    assert QN < 32768, QN
    NBLK = shard // 128
    NCHUNK = NBLK // CH
    assert NCHUNK * CH == NBLK

    dst = np.asarray(dst, dtype=np.int64)
    E = len(dst)
    deg = np.bincount(dst, minlength=N).astype(np.float64)
    recip = (1.0 / np.maximum(deg, 1.0)).astype(np.float32)

    core = dst // shard
    bl = (dst % shard) >> 7
    slot_val = (dst & 127).astype(np.float32)
    rval_val = recip[dst]
    ch = bl // CH
    bb = bl % CH

    # quadrant depends on per-core src table row; compute per core
    per_core = []
    CQ = cfg["cq_min"]
    # first pass: find global CQ
    cell_list = []
    for c in range(n_cores):
        m = core == c
        st = src_t_per_core[c][m]
        q = st // QN
        cell = (ch[m] * 4 + q) * CH + bb[m]
        cell_list.append((m, st, q, cell))
        cnts = np.bincount(cell, minlength=NCHUNK * 4 * CH)
        if len(cnts):
            CQ = max(CQ, int(math.ceil(cnts.max() / 128)))
    CB = 4 * CQ
    G = CH * CQ  # 128-edge groups per (chunk, quadrant) gather call

    for c in range(n_cores):
        m, st, q, cell = cell_list[c]
        order = np.argsort(cell, kind="stable")
        cell_s = cell[order]
        n_cells = NCHUNK * 4 * CH
        starts = np.searchsorted(cell_s, np.arange(n_cells))
        j = np.arange(len(cell_s)) - starts[cell_s]  # position within cell
        st_s = st[order]
        q_s = q[order]
        ch_s = ch[m][order]
        bb_s = bb[m][order]
        bl_s = bl[m][order]
        loc_idx = (st_s % QN).astype(np.int16)
        sv = slot_val[m][order]
        rv = rval_val[m][order]

        # gather idx array: per call (ch, q), position J = bb*CQ*128 + j,
        # wrapped into 16 partitions, replicated 8x down the partition dim.
        idx_w = np.zeros((128, NCHUNK * 4 * G * 8), dtype=np.int16)
        J = bb_s * (CQ * 128) + j
        col = (ch_s * 4 + q_s) * (G * 8) + J // 16
        row = (J % 16).astype(np.int64)
        for g in range(8):
            idx_w[row + 16 * g, col] = loc_idx
        # slot / recip one-hot source arrays: [128, NBLK*CB]
        slot_w = np.full((128, NBLK * CB), 999.0, dtype=np.float32)
        rval_w = np.zeros((128, NBLK * CB), dtype=np.float32)
        colS = bl_s * CB + q_s * CQ + j // 128
        rowS = j % 128
        slot_w[rowS, colS] = sv
        rval_w[rowS, colS] = rv
        per_core.append(dict(idx_w=idx_w, slot_w=slot_w, rval_w=rval_w))
    return CQ, per_core


def _lin_bf16(w):
    """[out,in] fp32 -> lhsT layout [in,out] bf16."""
    return np.ascontiguousarray(np.asarray(w).T).astype(BF16_NP)


def _bias_col(b):
    return np.asarray(b, np.float32).reshape(-1, 1)


# ----------------------------------------------------------------------------
# Device program
# ----------------------------------------------------------------------------


def build_program(cfg, CQp, CQb, reps=1, skip=()):
    N, DA, DU, H, OUT = cfg["N"], cfg["DA"], cfg["DU"], cfg["H"], cfg["OUT"]
    n_cores, shard = cfg["n_cores"], cfg["shard"]
    CH, AGC = cfg["CH"], cfg["AGC"]
    NPAD = n_cores * shard
    QN = NPAD // 4
    NBLK = shard // 128
    NCHUNK = NBLK // CH
    NAG = NCHUNK // AGC
    assert NAG * AGC == NCHUNK
    RPA = CH * AGC * 128  # rows per AG chunk (per core)
    BPA = CH * AGC        # blocks per AG chunk
    DA_PAD = ((DA + 15) // 16) * 16  # 304
    KA = [(k, min(128, DA_PAD - k)) for k in range(0, DA_PAD, 128)]
    TW = 512
    assert NPAD % TW == 0
    NT_A = NPAD // TW  # full-table article proj tiles
    n_tw = [(t, min(TW, shard - t)) for t in range(0, shard, TW)]
    n_own_full = shard // TW  # full tiles inside own shard
    own_rem = shard % TW
    Gp = CH * CQp
    Gb = CH * CQb
    CBp, CBb = 4 * CQp, 4 * CQb

    nc = bacc.Bacc("TRN2", debug=False)

    # ---- I/O ----
    xaT = nc.dram_tensor("xaT", [DA_PAD, NPAD], BF16, kind="ExternalInput")
    xuT = nc.dram_tensor("xuT", [DU, shard], BF16, kind="ExternalInput")
    w_in_aT = nc.dram_tensor("w_in_aT", [DA_PAD, H], BF16, kind="ExternalInput")
    b_in_a = nc.dram_tensor("b_in_a", [H, 1], FP32, kind="ExternalInput")
    w_in_uT = nc.dram_tensor("w_in_uT", [DU, H], BF16, kind="ExternalInput")
    b_in_u = nc.dram_tensor("b_in_u", [H, 1], FP32, kind="ExternalInput")
    convw = {}
    for et in ("c1p", "c1b", "c2p"):
        convw[et] = (
            nc.dram_tensor(f"{et}_wlT", [H, H], BF16, kind="ExternalInput"),
            nc.dram_tensor(f"{et}_bl", [H, 1], FP32, kind="ExternalInput"),
            nc.dram_tensor(f"{et}_wrT", [H, H], BF16, kind="ExternalInput"),
        )
    w_outT = nc.dram_tensor("w_outT", [H, OUT], BF16, kind="ExternalInput")
    b_out = nc.dram_tensor("b_out", [OUT, 1], FP32, kind="ExternalInput")
    iota_in = nc.dram_tensor("iota", [128, 128], BF16, kind="ExternalInput")
    ident_in = nc.dram_tensor("ident", [128, 128], BF16, kind="ExternalInput")
    idx_p = nc.dram_tensor("idx_p", [128, NCHUNK * 4 * Gp * 8], I16, kind="ExternalInput")
    slot_p = nc.dram_tensor("slot_p", [128, NBLK * CBp], FP32, kind="ExternalInput")
    rval_p = nc.dram_tensor("rval_p", [128, NBLK * CBp], FP32, kind="ExternalInput")
    idx_b = nc.dram_tensor("idx_b", [128, NCHUNK * 4 * Gb * 8], I16, kind="ExternalInput")
    slot_b = nc.dram_tensor("slot_b", [128, NBLK * CBb], FP32, kind="ExternalInput")
    rval_b = nc.dram_tensor("rval_b", [128, NBLK * CBb], FP32, kind="ExternalInput")
    out_d = nc.dram_tensor("out", [OUT, shard], FP32, kind="ExternalOutput")

    # internal HBM
    atab = nc.dram_tensor("atab", [NPAD, 128], BF16)  # cols 0:64 = a (rotated)
    comb_shard = [
        nc.dram_tensor(f"comb_shard{k}", [RPA, 128], BF16) for k in range(NAG)
    ]
    comb_rm = nc.dram_tensor("comb_rm", [NPAD, 128], BF16, addr_space="Shared")
    groups = [list(range(n_cores))]

    from contextlib import ExitStack

    with tile.TileContext(nc) as tc, ExitStack() as _stack:
        cpool = _stack.enter_context(tc.tile_pool(name="const", bufs=1))
        iota_sb = cpool.tile([128, 128], BF16, tag="iota")
        ident_sb = cpool.tile([128, 128], BF16, tag="ident")
        nc.sync.dma_start(iota_sb[:], iota_in[:])
        nc.sync.dma_start(ident_sb[:], ident_in[:])

        def load_const(t, shape, dtype, tag):
            s = cpool.tile(shape, dtype, tag=tag)
            nc.sync.dma_start(s[:], t[:])
            return s

        w_in_aT_s = cpool.tile([128, len(KA), H], BF16, tag="w_in_aT")
        for ki, (k0, kn) in enumerate(KA):
            nc.sync.dma_start(w_in_aT_s[0:kn, ki, :], w_in_aT[k0 : k0 + kn, :])
        b_in_a_s = load_const(b_in_a, [H, 1], FP32, "b_in_a")
        w_in_uT_s = load_const(w_in_uT, [DU, H], BF16, "w_in_uT")
        b_in_u_s = load_const(b_in_u, [H, 1], FP32, "b_in_u")
        convw_s = {}
        for et in ("c1p", "c1b", "c2p"):
            wlT, bl, wrT = convw[et]
            convw_s[et] = (
                load_const(wlT, [H, H], BF16, f"{et}_wlT"),
                load_const(bl, [H, 1], FP32, f"{et}_bl"),
                load_const(wrT, [H, H], BF16, f"{et}_wrT"),
            )
        w_outT_s = load_const(w_outT, [H, OUT], BF16, "w_outT")
        b_out_s = load_const(b_out, [OUT, 1], FP32, "b_out")
        slot_p_s = load_const(slot_p, [128, NBLK * CBp], FP32, "slot_p")
        rval_p_s = load_const(rval_p, [128, NBLK * CBp], FP32, "rval_p")
        slot_b_s = load_const(slot_b, [128, NBLK * CBb], FP32, "slot_b")
        rval_b_s = load_const(rval_b, [128, NBLK * CBb], FP32, "rval_b")

        # resident feature-major node tables (own shard)
        uT_own = cpool.tile([H, shard], BF16, tag="uT_own")
        aT_own = cpool.tile([H, shard], BF16, tag="aT_own")

        # ------------------- stage 1: input projections -------------------
        def _inproj():
          with (
            tc.tile_pool(name="ip_ps", bufs=3, space="PSUM") as ip_ps,
            tc.tile_pool(name="tp_ps", bufs=2, space="PSUM") as tp_ps,
            tc.tile_pool(name="ip_sb", bufs=6) as ip_sb,
            tc.tile_pool(name="rel_sb", bufs=3) as rel_sb,
            tc.tile_pool(name="tp_sb", bufs=3) as tp_sb,
          ):
            # ---- user proj (own shard) ----
            for t0, tw in n_tw:
                xt = ip_sb.tile([DU, TW], BF16, tag="xu")
                nc.sync.dma_start(xt[:, 0:tw], xuT[:, t0 : t0 + tw])
                ps = ip_ps.tile([H, TW], FP32, tag="ipps")
                nc.tensor.matmul(ps[:, 0:tw], w_in_uT_s[:], xt[:, 0:tw])
                nc.scalar.activation(
                    uT_own[:, t0 : t0 + tw], ps[:, 0:tw], AF.Relu, bias=b_in_u_s[:]
                )
            # u -> comb_shard cols 0:64 (transposed, per block)
            for b in range(NBLK):
                tp = tp_ps.tile([128, H], BF16, tag="tpu")
                nc.tensor.transpose(
                    tp[:], uT_own[:, b * 128 : (b + 1) * 128], ident_sb[0:H, 0:H]
                )
                st = tp_sb.tile([128, H], BF16, tag="stu")
                nc.scalar.copy(st[:], tp[:])
                k, rb = b // BPA, b % BPA
                nc.sync.dma_start(
                    comb_shard[k][rb * 128 : (rb + 1) * 128, 0:H], st[:]
                )
            # ---- article proj (FULL table, rotated cols; own shard first) ----
            for t in range(NT_A):
                ps = ip_ps.tile([H, TW], FP32, tag="ipps")
                for ki, (k0, kn) in enumerate(KA):
                    xt = ip_sb.tile([128, TW], BF16, tag="xa")
                    nc.sync.dma_start(
                        xt[0:kn, :], xaT[k0 : k0 + kn, t * TW : (t + 1) * TW]
                    )
                    nc.tensor.matmul(
                        ps[:],
                        w_in_aT_s[0:kn, ki, :],
                        xt[0:kn, :],
                        start=(ki == 0),
                        stop=(ki == len(KA) - 1),
                    )
                rel = rel_sb.tile([H, TW], BF16, tag="rel")
                nc.scalar.activation(rel[:], ps[:], AF.Relu, bias=b_in_a_s[:])
                if t < n_own_full:
                    nc.vector.tensor_copy(aT_own[:, t * TW : (t + 1) * TW], rel[:])
                elif own_rem and t == n_own_full:
                    nc.vector.tensor_copy(
                        aT_own[:, n_own_full * TW : shard], rel[:, 0:own_rem]
                    )
                tp = tp_ps.tile([128, 4 * H], BF16, tag="tpa")
                for jj in range(4):
                    nc.tensor.transpose(
                        tp[:, jj * H : (jj + 1) * H],
                        rel[:, jj * 128 : (jj + 1) * 128],
                        ident_sb[0:H, 0:H],
                    )
                st = tp_sb.tile([128, 4 * H], BF16, tag="sta")
                nc.scalar.copy(st[:], tp[:])
                for jj in range(4):
                    r0 = t * TW + jj * 128
                    nc.sync.dma_start(
                        atab[r0 : r0 + 128, 0:H], st[:, jj * H : (jj + 1) * H]
                    )

        # ------------------- conv layers -------------------
        def conv_layer(pools, cpools, gtable, idx_dram, slot_s, rval_s, CQ, fused):
            """fused=False: c1b (users): u1 -> comb_shard cols 64:128 +
            chunked AG issue. fused=True: c1p + c2p + head -> out."""
            CB = 4 * CQ
            G = CH * CQ
            (msg_p, s_p, agg_ps, lin_ps, agg_sb, outb_p, idx_pool) = pools
            if fused:
                (hd_sb,) = cpools
            else:
                ctp_ps, ctp_sb = cpools
            if fused:
                wlT1, bl1, wrT1 = convw_s["c1p"]
                wlT2, bl2, wrT2 = convw_s["c2p"]
            else:
                wlT1, bl1, wrT1 = convw_s["c1b"]
            for ch in range(NCHUNK):
                it = idx_pool.tile([128, 4 * G * 8], I16, tag="idxs")
                nc.sync.dma_start(
                    it[:], idx_dram[:, ch * 4 * G * 8 : (ch + 1) * 4 * G * 8]
                )
                msg = msg_p.tile([128, 4, G, 128], BF16, tag="msg")
                if "gather" not in skip:
                    # <=1024 idx (64 descs/lane) per call: single_packet mode
                    GS = max(d for d in range(1, 9) if G % d == 0)
                    for q in range(4):
                        for s in range(G // GS):
                            nc.gpsimd.dma_gather(
                                msg[:, q, s * GS : (s + 1) * GS, :],
                                gtable[q * QN : (q + 1) * QN, :],
                                it[:, (q * G + s * GS) * 8 : (q * G + (s + 1) * GS) * 8],
                                GS * 128,
                                GS * 128,
                                128,
                            )
                else:  # timing variant: every logical tile needs a write
                    nc.vector.memset(msg[:], 0.0)
                for bb in range(CH):
                    b = ch * CH + bb
                    # fused: one [128,128] chain, rows 0:H = c1p agg (u),
                    # rows H:2H = c2p agg (u1) — halves PE instruction count.
                    # c1b uses only rows 0:H of the same pool tag.
                    aggc = agg_ps.tile([2 * H, 128], FP32, tag="aggc")
                    agg1 = aggc[0:H, :]
                    first, last = True, False
                    for q in range(4):
                        for tq in range(CQ):
                            col = b * CB + q * CQ + tq
                            last = q == 3 and tq == CQ - 1
                            S = s_p.tile([128, 128], BF16, tag="S")
                            nc.vector.tensor_scalar(
                                S[:],
                                iota_sb[:],
                                slot_s[:, col : col + 1],
                                rval_s[:, col : col + 1],
                                ALU.is_equal,
                                ALU.mult,
                            )
                            if fused:
                                nc.tensor.matmul(
                                    aggc[:],
                                    msg[:, q, bb * CQ + tq, :],
                                    S[:],
                                    start=first,
                                    stop=last,
                                )
                            else:
                                nc.tensor.matmul(
                                    agg1,
                                    msg[:, q, bb * CQ + tq, 0:H],
                                    S[:],
                                    start=first,
                                    stop=last,
                                )
                            first = False
                    aggs1 = agg_sb.tile([H, 128], BF16, tag="aggs1")
                    nc.scalar.copy(aggs1[:], aggc[0:H, :])
                    lin1 = lin_ps.tile([H, 128], FP32, tag="lin")
                    nc.tensor.matmul(lin1[:], wlT1[:], aggs1[:], start=True, stop=False)
                    xdst = aT_own if fused else uT_own
                    nc.tensor.matmul(
                        lin1[:],
                        wrT1[:],
                        xdst[:, b * 128 : (b + 1) * 128],
                        start=False,
                        stop=True,
                    )
                    o1 = outb_p.tile([H, 128], BF16, tag="o1")
                    nc.scalar.activation(o1[:], lin1[:], AF.Relu, bias=bl1[:])
                    if not fused:
                        # u1 block -> comb_shard cols 64:128 (transposed)
                        tp = ctp_ps.tile([128, H], BF16, tag="ctp")
                        nc.tensor.transpose(tp[:], o1[:], ident_sb[0:H, 0:H])
                        st = ctp_sb.tile([128, H], BF16, tag="cst")
                        nc.scalar.copy(st[:], tp[:])
                        k, rb = b // BPA, b % BPA
                        nc.sync.dma_start(
                            comb_shard[k][rb * 128 : (rb + 1) * 128, H : 2 * H],
                            st[:],
                        )
                    else:
                        aggs2 = agg_sb.tile([H, 128], BF16, tag="aggs2")
                        nc.scalar.copy(aggs2[:], aggc[H : 2 * H, :])
                        lin2 = lin_ps.tile([H, 128], FP32, tag="lin")
                        nc.tensor.matmul(
                            lin2[:], wlT2[:], aggs2[:], start=True, stop=False
                        )
                        nc.tensor.matmul(
                            lin2[:], wrT2[:], o1[:], start=False, stop=True
                        )
                        a2 = outb_p.tile([H, 128], BF16, tag="a2")
                        nc.vector.tensor_scalar_add(a2[:], lin2[:], bl2[:])
                        hp = lin_ps.tile([H, 128], FP32, tag="lin")
                        nc.tensor.matmul(hp[0:OUT, :], w_outT_s[:], a2[:])
                        ho = hd_sb.tile([OUT, 128], FP32, tag="hdo")
                        nc.vector.tensor_scalar_add(ho[:], hp[0:OUT, :], b_out_s[:])
                        nc.sync.dma_start(out_d[:, b * 128 : (b + 1) * 128], ho[:])
                if not fused and ch % AGC == AGC - 1 and "ag" not in skip:
                    k = ch // AGC
                    nc.gpsimd.collective_compute(
                        "AllGather",
                        ALU.bypass,
                        replica_groups=groups,
                        ins=[comb_shard[k][:]],
                        outs=[comb_rm[k * n_cores * RPA : (k + 1) * n_cores * RPA, :]],
                    )

        def _convs():
          with (
            tc.tile_pool(name="msg", bufs=2) as msg_p,
            tc.tile_pool(name="S", bufs=4) as s_p,
            tc.tile_pool(name="agg_ps", bufs=2, space="PSUM") as agg_ps,
            tc.tile_pool(name="lin_ps", bufs=3, space="PSUM") as lin_ps,
            tc.tile_pool(name="agg_sb", bufs=4) as agg_sb,
            tc.tile_pool(name="outb", bufs=4) as outb_p,
            tc.tile_pool(name="idxs", bufs=3) as idx_pool,
          ):
            pools = (msg_p, s_p, agg_ps, lin_ps, agg_sb, outb_p, idx_pool)
            # users conv first (produces u1, issues chunked AllGathers)
            with (
                tc.tile_pool(name="ctp_ps", bufs=2, space="PSUM") as ctp_ps,
                tc.tile_pool(name="ctp_sb", bufs=3) as ctp_sb,
            ):
                conv_layer(pools, (ctp_ps, ctp_sb), atab, idx_b,
                           slot_b_s, rval_b_s, CQb, False)
            # fused c1p + c2p + head over article dsts
            with tc.tile_pool(name="hd_sb", bufs=3) as hd_sb:
                conv_layer(pools, (hd_sb,), comb_rm, idx_p,
                           slot_p_s, rval_p_s, CQp, True)

        for _rep in range(reps):
            _inproj()
            if "convs" not in skip:
                _convs()

    nc.compile()
    return nc


# ----------------------------------------------------------------------------
# Entry point
# ----------------------------------------------------------------------------

_CACHE = {}


def build_in_maps(inputs, cfg, CQp, per_core_p, CQb, per_core_b):
    N, DA, DU, H = cfg["N"], cfg["DA"], cfg["DU"], cfg["H"]
    n_cores, shard = cfg["n_cores"], cfg["shard"]
    NPAD = n_cores * shard
    DA_PAD = ((DA + 15) // 16) * 16
    xa = np.asarray(inputs["x_article"], np.float32)
    xu = np.asarray(inputs["x_user"], np.float32)

    shared = dict(
        w_in_aT=np.concatenate(
            [_lin_bf16(inputs["w_in_a"]), np.zeros((DA_PAD - DA, H), BF16_NP)], 0
        ),
        b_in_a=_bias_col(inputs["b_in_a"]),
        w_in_uT=_lin_bf16(inputs["w_in_u"]),
        b_in_u=_bias_col(inputs["b_in_u"]),
        w_outT=_lin_bf16(inputs["w_out"]),
        b_out=_bias_col(inputs["b_out"]),
        iota=np.tile(np.arange(128), (128, 1)).astype(BF16_NP),
        ident=np.eye(128, dtype=BF16_NP),
    )
    for et in ("c1p", "c1b", "c2p"):
        shared[f"{et}_wlT"] = _lin_bf16(inputs[f"{et}_wl"])
        shared[f"{et}_bl"] = _bias_col(inputs[f"{et}_bl"])
        shared[f"{et}_wrT"] = _lin_bf16(inputs[f"{et}_wr"])

    # full article feature table, feature-major, padded
    xaT_nat = np.zeros((DA_PAD, NPAD), BF16_NP)
    xaT_nat[:DA, :N] = xa.T.astype(BF16_NP)

    in_maps = []
    for c in range(n_cores):
        c0, c1 = c * shard, min((c + 1) * shard, N)
        xuT_c = np.zeros((DU, shard), BF16_NP)
        xuT_c[:, : c1 - c0] = xu[c0:c1].T.astype(BF16_NP)
        m = dict(shared)
        m["xaT"] = np.roll(xaT_nat, -c * shard, axis=1)
        m["xuT"] = xuT_c
        m["idx_p"] = per_core_p[c]["idx_w"]
        m["slot_p"] = per_core_p[c]["slot_w"]
        m["rval_p"] = per_core_p[c]["rval_w"]
        m["idx_b"] = per_core_b[c]["idx_w"]
        m["slot_b"] = per_core_b[c]["slot_w"]
        m["rval_b"] = per_core_b[c]["rval_w"]
        in_maps.append(m)
    return in_maps


def _prep_all(inputs, cfg):
    n_cores, shard = cfg["n_cores"], cfg["shard"]
    NPAD = n_cores * shard
    # posts: user -> article. src users, table row = AG-chunk layout.
    src_p = np.asarray(inputs["ei_posts"][0], np.int64)
    dst_p = np.asarray(inputs["ei_posts"][1], np.int64)
    trow_p = user_trow(src_p, cfg)
    CQp, per_core_p = prep_edges([trow_p] * n_cores, dst_p, cfg)
    # posted_by: article -> user. src articles, per-core rotated rows.
    src_b = np.asarray(inputs["ei_pb"][0], np.int64)
    dst_b = np.asarray(inputs["ei_pb"][1], np.int64)
    trows_b = [(src_b - c * shard) % NPAD for c in range(n_cores)]
    CQb, per_core_b = prep_edges(trows_b, dst_b, cfg)
    return CQp, per_core_p, CQb, per_core_b


def _run(inputs, cfg, trace=False, reps=1):
    N, n_cores, shard = cfg["N"], cfg["n_cores"], cfg["shard"]

    CQp, per_core_p, CQb, per_core_b = _prep_all(inputs, cfg)

    key = (tuple(sorted(cfg.items())), CQp, CQb, reps)
    if key not in _CACHE:
        _CACHE[key] = build_program(cfg, CQp, CQb, reps)
    nc = _CACHE[key]

    in_maps = build_in_maps(inputs, cfg, CQp, per_core_p, CQb, per_core_b)

    res = run_bass_kernel_spmd(nc, in_maps, list(range(n_cores)), trace=trace)
    outs = [res.results[c]["out"] for c in range(n_cores)]  # [2, shard] each
    full = np.concatenate(outs, axis=1)[:, :N].T.astype(np.float32)
    return np.ascontiguousarray(full), res


def kernel(**inputs):
    out, _ = _run(inputs, full_cfg(), trace=False)
    return out
